# revision 1
# baseline (speedup 1.0000x reference)
"""Chamfer + KL loss on 8 Trainium2 NeuronCores.

Strategy (data-parallel over batch, 2 batches per core):
  For each batch and each orientation (gts-stationary / preds-stationary),
  compute Q[n, m] = ||mov_m||^2 - 2 <stat_n, mov_m> via TensorE matmuls and
  take min over m with VectorE reduce_min.  Then
      min_m P[n, m] = ||stat_n||^2 + min_m Q[n, m],
  so summing mins plus the stationary squared-norm total gives each Chamfer
  direction.  The matmul contraction carries the coordinates in bf16 hi/lo
  split form (K=11: 3x hi*hi, 3x hi*lo, 3x lo*hi, plus a 2-term bf16 split
  of ||mov||^2 against ones), giving ~fp32-level accuracy with full-rate
  bf16 PE throughput.  The 4 N=512 matmuls of each [128, 2048] PSUM tile
  are packed into the four 32-row groups of the PE array (tile_position)
  so they run concurrently.  Per-core partial sums are combined on host.
"""
import sys

sys.path.insert(0, "/opt/trn_rl_repo")

import numpy as np
import jax
from jax.sharding import Mesh, PartitionSpec
from jax.experimental.shard_map import shard_map

import concourse.bacc as bacc
import concourse.tile as tile
import concourse.mybir as mybir
from concourse.bass2jax import (
    _bass_exec_p,
    install_neuronx_cc_hook,
    partition_id_tensor,
)

F32 = mybir.dt.float32
BF16 = mybir.dt.bfloat16

N_CORES = 8
B_PER_CORE = 2
NPTS = 4096
NBLK = 32          # stationary 128-blocks per cloud
NJ = 2             # moving 2048-col halves
N_UNITS = B_PER_CORE * 2 * NBLK * NJ  # 256
Z = 128


def _build_nc():
    nc = bacc.Bacc("TRN2", target_bir_lowering=False, debug=False)
    gts_d = nc.dram_tensor("gts_c", [B_PER_CORE, 3, NPTS], F32, kind="ExternalInput")
    preds_d = nc.dram_tensor("preds_c", [B_PER_CORE, 3, NPTS], F32, kind="ExternalInput")
    mu_d = nc.dram_tensor("mu_c", [B_PER_CORE, Z], F32, kind="ExternalInput")
    lv_d = nc.dram_tensor("logvar_c", [B_PER_CORE, Z], F32, kind="ExternalInput")
    R_out = nc.dram_tensor("R_out", [128, N_UNITS], F32, kind="ExternalOutput")
    sums_out = nc.dram_tensor("sums_out", [128, 4], F32, kind="ExternalOutput")
    kl_out = nc.dram_tensor("kl_out", [B_PER_CORE, 1], F32, kind="ExternalOutput")

    src_d = {0: gts_d, 1: preds_d}  # tensor id -> dram handle

    with tile.TileContext(nc) as tc:
        with (
            tc.tile_pool(name="sb", bufs=1) as sb,
            tc.tile_pool(name="dramp", bufs=8, space="DRAM") as dramp,
            tc.tile_pool(name="ps", bufs=2, space="PSUM") as ps,
        ):
            # ---- staging: all 4 coordinate matrices as [12, 4096] f32
            stage = sb.tile([12, NPTS], F32, tag="stage")
            for b in range(B_PER_CORE):
                nc.sync.dma_start(stage[6 * b + 0 : 6 * b + 3, :], gts_d[b])
                nc.sync.dma_start(stage[6 * b + 3 : 6 * b + 6, :], preds_d[b])

            # row range of tensor t=(b, which): which 0=gts 1=preds
            def rows(b, which):
                r0 = 6 * b + 3 * which
                return r0, r0 + 3

            # ---- split forms (all [12, 4096])
            m2 = sb.tile([12, NPTS], F32, tag="m2")
            nc.vector.tensor_scalar_mul(m2[:], stage[:], -2.0)
            sh = sb.tile([12, NPTS], BF16, tag="sh")
            nc.vector.tensor_copy(sh[:], m2[:])
            sl = sb.tile([12, NPTS], BF16, tag="sl")
            nc.vector.tensor_tensor(out=sl[:], in0=m2[:], in1=sh[:], op=mybir.AluOpType.subtract)
            mh = sb.tile([12, NPTS], BF16, tag="mh")
            nc.vector.tensor_copy(mh[:], stage[:])
            ml = sb.tile([12, NPTS], BF16, tag="ml")
            nc.vector.tensor_tensor(out=ml[:], in0=stage[:], in1=mh[:], op=mybir.AluOpType.subtract)

            ones2 = sb.tile([2, NPTS], BF16, tag="ones2")
            nc.vector.memset(ones2[:], 1.0)

            # ---- squared norms in [128, 32] layout + bf16 hi/lo rows via DRAM bounce
            sums_t = sb.tile([128, 4], F32, tag="sums_t")
            # norm_rows[(b, which)] = (dram_hi, dram_lo) flattened [1, 4096] sources
            norm_rows = {}
            for b in range(B_PER_CORE):
                for which in range(2):
                    d0 = src_d[which]
                    ct = []
                    for d in range(3):
                        cd = sb.tile([128, 32], F32, tag=f"c{d}")
                        nc.sync.dma_start(
                            cd[:], d0[b, d].rearrange("(p c) -> p c", p=128)
                        )
                        ct.append(cd)
                    s0 = sb.tile([128, 32], F32, tag="s0")
                    t0 = sb.tile([128, 32], F32, tag="t0")
                    nc.vector.tensor_tensor(out=s0[:], in0=ct[0][:], in1=ct[0][:], op=mybir.AluOpType.mult)
                    nc.vector.tensor_tensor(out=t0[:], in0=ct[1][:], in1=ct[1][:], op=mybir.AluOpType.mult)
                    nc.vector.tensor_tensor(out=s0[:], in0=s0[:], in1=t0[:], op=mybir.AluOpType.add)
                    nc.vector.tensor_tensor(out=t0[:], in0=ct[2][:], in1=ct[2][:], op=mybir.AluOpType.mult)
                    nc.vector.tensor_tensor(out=s0[:], in0=s0[:], in1=t0[:], op=mybir.AluOpType.add)
                    # partial sum of norms for host
                    nc.vector.tensor_reduce(
                        sums_t[:, 2 * b + which : 2 * b + which + 1],
                        s0[:],
                        axis=mybir.AxisListType.X,
                        op=mybir.AluOpType.add,
                    )
                    rh = sb.tile([128, 32], BF16, tag="rh")
                    nc.vector.tensor_copy(rh[:], s0[:])
                    rl = sb.tile([128, 32], BF16, tag="rl")
                    nc.vector.tensor_tensor(out=rl[:], in0=s0[:], in1=rh[:], op=mybir.AluOpType.subtract)
                    bh = dramp.tile([128, 32], BF16, tag="bh")
                    bl = dramp.tile([128, 32], BF16, tag="bl")
                    nc.sync.dma_start(bh[:], rh[:])
                    nc.sync.dma_start(bl[:], rl[:])
                    norm_rows[(b, which)] = (bh, bl)
            nc.sync.dma_start(sums_out[:], sums_t[:])

            # ---- assemble stationary / moving feature tensors [128, 4096] bf16
            # orientation o: 0 -> stat=gts, mov=preds ; 1 -> stat=preds, mov=gts
            stat_all = {}
            mov_all = {}
            for b in range(B_PER_CORE):
                for o in range(2):
                    st_w, mv_w = (0, 1) if o == 0 else (1, 0)
                    r0, r1 = rows(b, st_w)
                    sa = sb.tile([128, NPTS], BF16, tag=f"stat{b}{o}")
                    nc.sync.dma_start(sa[0:3, :], sh[r0:r1, :])
                    nc.sync.dma_start(sa[3:6, :], sh[r0:r1, :])
                    nc.sync.dma_start(sa[6:9, :], sl[r0:r1, :])
                    nc.sync.dma_start(sa[9:11, :], ones2[:])
                    m0, m1 = rows(b, mv_w)
                    ma = sb.tile([128, NPTS], BF16, tag=f"mov{b}{o}")
                    nc.sync.dma_start(ma[0:3, :], mh[m0:m1, :])
                    nc.sync.dma_start(ma[3:6, :], ml[m0:m1, :])
                    nc.sync.dma_start(ma[6:9, :], mh[m0:m1, :])
                    bh, bl = norm_rows[(b, mv_w)]
                    nc.sync.dma_start(ma[9:10, :], bh[:].rearrange("p c -> (p c)")[None, :])
                    nc.sync.dma_start(ma[10:11, :], bl[:].rearrange("p c -> (p c)")[None, :])
                    for g in range(1, 4):
                        nc.sync.dma_start(sa[32 * g : 32 * g + 11, :], sa[0:11, :])
                        nc.sync.dma_start(ma[32 * g : 32 * g + 11, :], ma[0:11, :])
                    stat_all[(b, o)] = sa
                    mov_all[(b, o)] = ma

            # ---- KL pieces: t_b = sum_z (logvar - mu^2 - exp(logvar))
            mu_t = sb.tile([B_PER_CORE, Z], F32, tag="mu_t")
            lv_t = sb.tile([B_PER_CORE, Z], F32, tag="lv_t")
            nc.sync.dma_start(mu_t[:], mu_d[:])
            nc.sync.dma_start(lv_t[:], lv_d[:])
            msq = sb.tile([B_PER_CORE, Z], F32, tag="msq")
            nc.vector.tensor_tensor(out=msq[:], in0=mu_t[:], in1=mu_t[:], op=mybir.AluOpType.mult)
            ex = sb.tile([B_PER_CORE, Z], F32, tag="ex")
            nc.scalar.activation(ex[:], lv_t[:], mybir.ActivationFunctionType.Exp)
            kt = sb.tile([B_PER_CORE, Z], F32, tag="kt")
            nc.vector.tensor_tensor(out=kt[:], in0=lv_t[:], in1=msq[:], op=mybir.AluOpType.subtract)
            nc.vector.tensor_tensor(out=kt[:], in0=kt[:], in1=ex[:], op=mybir.AluOpType.subtract)
            kl_t = sb.tile([B_PER_CORE, 1], F32, tag="kl_t")
            nc.vector.tensor_reduce(kl_t[:], kt[:], axis=mybir.AxisListType.X, op=mybir.AluOpType.add)
            nc.sync.dma_start(kl_out[:], kl_t[:])

            # ---- main loop
            R = sb.tile([128, N_UNITS], F32, tag="R")
            for b in range(B_PER_CORE):
                for o in range(2):
                    sa = stat_all[(b, o)]
                    ma = mov_all[(b, o)]
                    for i in range(NBLK):
                        for j in range(NJ):
                            u = ((b * 2 + o) * NBLK + i) * NJ + j
                            Pe = ps.tile([128, 4 * 512], F32)
                            for s in range(4):
                                g = 32 * s
                                nc.tensor.matmul(
                                    Pe[:, s * 512 : (s + 1) * 512],
                                    sa[g : g + 11, i * 128 : (i + 1) * 128],
                                    ma[g : g + 11, j * 2048 + s * 512 : j * 2048 + (s + 1) * 512],
                                    start=True,
                                    stop=True,
                                    tile_position=(g, 0),
                                )
                            nc.vector.tensor_reduce(
                                R[:, u : u + 1],
                                Pe[:],
                                axis=mybir.AxisListType.X,
                                op=mybir.AluOpType.min,
                            )
            nc.sync.dma_start(R_out[:], R[:])
    nc.compile()
    return nc


class _Runner:
    def __init__(self, nc, n_cores):
        install_neuronx_cc_hook()
        self.n_cores = n_cores
        partition_name = nc.partition_id_tensor.name if nc.partition_id_tensor else None
        in_names, out_names, out_avals, zero_outs = [], [], [], []
        for alloc in nc.m.functions[0].allocations:
            if not isinstance(alloc, mybir.MemoryLocationSet):
                continue
            name = alloc.memorylocations[0].name
            if alloc.kind == "ExternalInput":
                if name != partition_name:
                    in_names.append(name)
            elif alloc.kind == "ExternalOutput":
                out_names.append(name)
                shape = tuple(alloc.tensor_shape)
                dtype = mybir.dt.np(alloc.dtype)
                out_avals.append(jax.core.ShapedArray(shape, dtype))
                zero_outs.append(np.zeros(shape, dtype))
        self.in_names, self.out_names = in_names, out_names
        self.out_avals, self.zero_outs = out_avals, zero_outs
        n_params, n_outs = len(in_names), len(out_names)
        self.n_params = n_params
        all_in = list(in_names) + list(out_names)
        if partition_name is not None:
            all_in.append(partition_name)
        donate = tuple(range(n_params, n_params + n_outs))

        def _body(*args):
            operands = list(args)
            if partition_name is not None:
                operands.append(partition_id_tensor())
            return tuple(
                _bass_exec_p.bind(
                    *operands,
                    out_avals=tuple(out_avals),
                    in_names=tuple(all_in),
                    out_names=tuple(out_names),
                    lowering_input_output_aliases=(),
                    sim_require_finite=True,
                    sim_require_nnan=True,
                    nc=nc,
                )
            )

        devices = jax.devices()[:n_cores]
        mesh = Mesh(np.asarray(devices), ("core",))
        in_specs = (PartitionSpec("core"),) * (n_params + n_outs)
        out_specs = (PartitionSpec("core"),) * n_outs
        self.fn = jax.jit(
            shard_map(_body, mesh=mesh, in_specs=in_specs, out_specs=out_specs,
                      check_rep=False),
            donate_argnums=donate,
            keep_unused=True,
        )

    def run(self, in_maps):
        n = self.n_cores
        per_core = [[np.asarray(m[k]) for k in self.in_names] for m in in_maps]
        concat_in = [
            np.concatenate([per_core[c][i] for c in range(n)], axis=0)
            for i in range(self.n_params)
        ]
        concat_zeros = [
            np.zeros((n * z.shape[0], *z.shape[1:]), z.dtype) for z in self.zero_outs
        ]
        outs = self.fn(*concat_in, *concat_zeros)
        return [
            {
                k: np.asarray(outs[i]).reshape(n, *self.out_avals[i].shape)[c]
                for i, k in enumerate(self.out_names)
            }
            for c in range(n)
        ]


_CACHE = {}


def _get_runner():
    if "r" not in _CACHE:
        nc = _build_nc()
        _CACHE["r"] = _Runner(nc, N_CORES)
    return _CACHE["r"]


def _combine(outs):
    """Host-side combine of per-core partials (float64)."""
    total = 0.0
    for c in range(N_CORES):
        R = outs[c]["R_out"].astype(np.float64)       # [128, 256]
        sums = outs[c]["sums_out"].astype(np.float64)  # [128, 4]
        kl_t = outs[c]["kl_out"].astype(np.float64)    # [2, 1]
        # R columns: u = ((b*2 + o)*32 + i)*2 + j
        Rr = R.reshape(128, B_PER_CORE, 2, NBLK, NJ)
        rmin = Rr.min(axis=4)              # min over j -> [128, b, o, i]
        mins_sum = rmin.sum(axis=(0, 3))   # [b, o]
        for b in range(B_PER_CORE):
            for o in range(2):
                st_w = 0 if o == 0 else 1  # stationary tensor: 0=gts, 1=preds
                stat_norm = sums[:, 2 * b + st_w].sum()
                total += mins_sum[b, o] + stat_norm
        # KL: -0.5 * sum(1 + lv - mu^2 - exp(lv)) = -0.5 * (Z + t_b) per batch
        for b in range(B_PER_CORE):
            total += -0.5 * (Z + kl_t[b, 0])
    return total


def kernel(preds, gts, mu, logvar):
    preds = np.asarray(preds, np.float32)
    gts = np.asarray(gts, np.float32)
    mu = np.asarray(mu, np.float32)
    logvar = np.asarray(logvar, np.float32)
    runner = _get_runner()
    in_maps = []
    for c in range(N_CORES):
        sl = slice(B_PER_CORE * c, B_PER_CORE * (c + 1))
        in_maps.append(
            {
                "gts_c": gts[sl],
                "preds_c": preds[sl],
                "mu_c": mu[sl],
                "logvar_c": logvar[sl],
            }
        )
    outs = runner.run(in_maps)
    return np.float32(_combine(outs))
